# revision 24
# baseline (speedup 1.0000x reference)
"""Causal multi-head attention (B=2, S=2048, H=32, D=128) on 8 TRN2 NeuronCores.

Strategy (tensor-parallel over (batch, head) pairs — 64 pairs, 8 per core):

Host side packs per-head inputs into device-friendly layouts:
  qT, kT : [hpc, D, S]  bf16 — Q^T / K^T per head (d on partitions)
  vA     : [hpc, 128, NT*129] bf16 — V tiled [kv-tile, 129] with a ones
           column appended (col 128) so the softmax denominator falls out of
           the PV matmul as an extra output column.
  tri    : [128, 128] bf16 — tri[p, f] = 1 iff p <= f (causal keep-mask for
           diagonal 128x128 blocks in S^T layout).

Device — one gapless bank-packed wave pipeline over the whole core:
  All causal S^T pieces (kv-tile x q-block, causally trimmed, split at
  512-col PSUM bank boundaries) of all 8 heads form a dense 272-bank
  stream, cut into uniform 3-bank (1536-col) waves; the only short waves
  (512 cols) are the very first and last, where the pipeline is filling or
  draining anyway.  Waves may straddle a head boundary (the exp is then
  split per head segment).  Per wave: QK matmuls into a double-buffered
  3-bank PSUM tile (PE, bf16), one exp per head-segment (ACT, scale folded
  in; no max subtraction — scores are O(5) so fp32 exp is safe) into a
  per-head pt tile in SBUF, diagonal squares fixed by a bf16 tri-mask
  multiply (DVE).  PV trails two waves behind with P^T chunks as the
  stationary operand, so output lands in [q, d] layout and the vA ones
  column accumulates row sums; per q-block the accumulator is drained
  (DVE) and normalized on the otherwise-idle Pool engine, then DMA'd out
  as bf16.

Uniform waves keep the PE->ACT 2-slot PSUM rotation cadence jitter-free:
ACT (the bottleneck at ~134us/core) sees near-identical 1536-col work
items back to back for the entire kernel.  Upper-triangle blocks are
skipped entirely: exp(-1e9) underflows to exactly 0.0 in fp32, so
dropping them is bit-equivalent to the reference softmax.
"""

import math

import numpy as np
import ml_dtypes

import concourse.bass as bass
import concourse.mybir as mybir
import concourse.tile as tile
from concourse import bacc
from concourse.tile_rust import add_dep_helper

B, S, H, D = 2, 2048, 32, 128
N_CORES = 8
HPC = (B * H) // N_CORES  # head-pairs per core
VW = D + 1                # V width including the ones column
SCALE = 1.0 / math.sqrt(D)
CHUNK_OFF = (0, 129, 258, 512)  # PV output chunk offsets (chunk 3 in bank 1)
BF16 = mybir.dt.bfloat16
F32 = mybir.dt.float32


def _head_layout(s=S):
    """Dense bank-packed causal S^T piece list for one head.

    Pieces are (j, qb, c0, nch, pos, square): kv-tile j of q-block qb,
    chunks c0..c0+nch-1 (128-col q-chunks within the q-block), packed at
    column pos.  `square` marks the piece whose first chunk is the diagonal
    square needing the tri mask.  Pieces never cross 512-col PSUM bank
    boundaries (split on the fly); widths/positions are all multiples of
    128 so chunks stay 128-aligned.
    """
    pieces = []
    pos = 0
    qnb = s // 512
    for qb in range(qnb):
        for j in range(4 * qb + 4):
            c0 = max(0, j - 4 * qb)
            nch = 4 - c0
            first = True
            while nch > 0:
                room = (512 - pos % 512) // 128
                n = min(nch, room)
                pieces.append((j, qb, c0, n, pos, first and c0 == j - 4 * qb))
                pos += n * 128
                c0 += n
                nch -= n
                first = False
    return pieces, pos


def build_module(hpc=HPC, s=S, wave_banks=3):
    nt = s // 128
    qnb = s // 512
    hpieces, head_cols = _head_layout(s)
    # Global piece stream: (h, j, qb, c0, nch, pos-within-head, square)
    gpieces = [(h, *p) for h in range(hpc) for p in hpieces]
    total_cols = hpc * head_cols
    # Wave cut points over global columns: short first wave (quick kernel
    # start), uniform 1536 interior waves, short remainder at the end.
    cuts = [0, 512]
    while cuts[-1] < total_cols:
        cuts.append(min(cuts[-1] + wave_banks * 512, total_cols))

    nc = bacc.Bacc(trn_type="TRN2")
    qT = nc.dram_tensor("qT", [hpc, D, s], BF16, kind="ExternalInput")
    kT = nc.dram_tensor("kT", [hpc, D, s], BF16, kind="ExternalInput")
    vA = nc.dram_tensor("vA", [hpc, 128, nt * VW], BF16, kind="ExternalInput")
    tri = nc.dram_tensor("tri", [128, 128], BF16, kind="ExternalInput")
    out = nc.dram_tensor("out", [hpc, 128, nt * D], BF16, kind="ExternalOutput")

    exp_fn = mybir.ActivationFunctionType.Exp

    last_piece = {}
    for idx, (h, j, qb, c0, nch, pos, sq) in enumerate(gpieces):
        last_piece[(h, qb)] = idx

    with tile.TileContext(nc) as tc:
        with (
            tc.tile_pool(name="const", bufs=1) as cpool,
            tc.tile_pool(name="io", bufs=2) as iopool,
            tc.tile_pool(name="pt", bufs=2) as ptpool,
            tc.tile_pool(name="ps", bufs=2, space="PSUM") as pspool,
            tc.tile_pool(name="po", bufs=1, space="PSUM") as popool,
            tc.tile_pool(name="nrm", bufs=4) as npool,
            tc.tile_pool(name="un", bufs=2) as unpool,
        ):
            tri_sb = cpool.tile([128, 128], BF16, tag="tri", name="tri_sb")
            head_st = {}
            qstate = {}
            pending = []   # wave dicts awaiting PV emission (lag queue)
            PV_LAG = 2     # PV trails scores by 2 waves: its exp/tri deps
                           # are complete by then (ps slot WAR), so PE never
                           # head-of-line blocks on ACT/DVE.

            def emit_head_dma(h):
                # Tiny first-wave slices first so head 0's first matmuls
                # start as soon as possible; 512-col slices keep later
                # waves' needs ahead of the bulk.
                kT_sb = iopool.tile([128, s], BF16, tag="kT", name=f"kT{h}")
                qT_sb = iopool.tile([128, s], BF16, tag="qT", name=f"qT{h}")
                vA_sb = iopool.tile([128, nt * VW], BF16, tag="vA",
                                    name=f"vA{h}")
                if h == 0:
                    # Parallel trigger engines: each dma_start costs ~600ns
                    # serial on its trigger engine's sequencer, so spreading
                    # the critical first slices across sync/gpsimd/vector
                    # gets wave 0/1 data in flight ~2us sooner.
                    nc.sync.dma_start(out=kT_sb[:, 0:512], in_=kT[h][:, 0:512])
                    nc.gpsimd.dma_start(out=qT_sb[:, 0:512],
                                        in_=qT[h][:, 0:512])
                    nc.gpsimd.dma_start(out=kT_sb[:, 512:1024],
                                        in_=kT[h][:, 512:1024])
                    nc.sync.dma_start(out=qT_sb[:, 512:1024],
                                      in_=qT[h][:, 512:1024])
                    nc.gpsimd.dma_start(out=vA_sb[:, 0:8 * VW],
                                        in_=vA[h][:, 0:8 * VW])
                    nc.scalar.dma_start(out=tri_sb, in_=tri[:, :])
                    nc.sync.dma_start(out=kT_sb[:, 1024:s],
                                      in_=kT[h][:, 1024:s])
                    nc.scalar.dma_start(out=qT_sb[:, 1024:s],
                                        in_=qT[h][:, 1024:s])
                    nc.gpsimd.dma_start(out=vA_sb[:, 8 * VW:],
                                        in_=vA[h][:, 8 * VW:])
                else:
                    nc.sync.dma_start(out=kT_sb, in_=kT[h])
                    nc.sync.dma_start(out=qT_sb, in_=qT[h])
                    nc.gpsimd.dma_start(out=vA_sb, in_=vA[h])
                out_sb = iopool.tile([128, nt * D], BF16, tag="osb",
                                     name=f"osb{h}")
                pt_sb = ptpool.tile([128, head_cols], BF16, tag="pt",
                                    name=f"pt{h}")
                head_st[h] = {"kT": kT_sb, "qT": qT_sb, "vA": vA_sb,
                              "out": out_sb, "pt": pt_sb}

            def emit_scores(wv):
                w0, w1 = wv["lo"], wv["hi"]
                ps = pspool.tile(
                    [128, wave_banks * 512], F32, tag="ps",
                    name=f"ps{wv['wi']}",
                )
                # High priority: the wave ACT consumes next must never
                # queue behind lagged-PV filler in the PE stream.
                with tc.high_priority(offset=150):
                    for (h, j, qb, c0, nch, pos, sq) in wv["pieces"]:
                        st = head_st[h]
                        w = nch * 128
                        q0 = qb * 512 + c0 * 128
                        g = h * head_cols + pos
                        nc.tensor.matmul(
                            ps[:, g - w0:g - w0 + w],
                            st["kT"][:, j * 128:(j + 1) * 128],
                            st["qT"][:, q0:q0 + w],
                            start=True, stop=True,
                        )
                # one exp per head segment of the wave
                for h, lo, hi in wv["segs"]:
                    nc.scalar.activation(
                        head_st[h]["pt"][:, lo:hi],
                        ps[:, h * head_cols + lo - w0:
                           h * head_cols + hi - w0],
                        exp_fn, scale=SCALE,
                    )
                for (h, j, qb, c0, nch, pos, sq) in wv["pieces"]:
                    if sq:  # diagonal square: causal mask
                        pt = head_st[h]["pt"]
                        nc.vector.tensor_mul(
                            pt[:, pos:pos + 128], pt[:, pos:pos + 128], tri_sb,
                        )

            def emit_pv(wv):
                for pi, (h, j, qb, c0, nch, pos, sq) in zip(
                        wv["pidx"], wv["pieces"]):
                    st = head_st[h]
                    key = (h, qb)
                    if key not in qstate:
                        # PV accumulator for this q-block.  start=True on
                        # the first write to each bank clears it bank-wide
                        # (has_written), so every other first write to the
                        # bank lands overwrite-where-clear and later ones
                        # accumulate.  Explicit deps pin the bank-0 clear
                        # before its sibling chunks' first writes.
                        qstate[key] = {
                            "po": popool.tile([128, 1024], F32, tag="po",
                                              name=f"po{h}_{qb}"),
                            "banks": set(), "clear": None, "written": set(),
                        }
                    q = qstate[key]
                    po = q["po"]
                    for c in range(c0, c0 + nch):
                        bank = 1 if c == 3 else 0
                        first_bank = bank not in q["banks"]
                        q["banks"].add(bank)
                        mm = nc.tensor.matmul(
                            po[:, CHUNK_OFF[c]:CHUNK_OFF[c] + VW],
                            st["pt"][:, pos + (c - c0) * 128:
                                     pos + (c - c0) * 128 + 128],
                            st["vA"][:, j * VW:(j + 1) * VW],
                            start=first_bank,
                            stop=False, skip_group_check=True,
                        )
                        if bank == 0:
                            if first_bank:
                                q["clear"] = mm.ins
                            elif c not in q["written"]:
                                add_dep_helper(
                                    mm.ins, q["clear"], sync=False,
                                    reason="bank0 clear before sibling writes")
                        q["written"].add(c)
                    if pi == last_piece[(h, qb)]:
                        emit_norm(h, qb, q["po"])

            def emit_norm(h, qb, po):
                st = head_st[h]
                if h == hpc - 1 and qb == qnb - 1:
                    # Kernel tail: normalize straight from PSUM on DVE
                    # (fast cadence, no drain — nothing reuses po after).
                    # Per-chunk out DMAs overlap each chunk's ~4us DMA
                    # trigger->transfer latency with the next normalize.
                    for c in range(4):
                        qi = qb * 4 + c
                        rc = npool.tile([128, 1], F32, tag="rc",
                                        name=f"rc{h}_{qi}")
                        nc.vector.reciprocal(
                            rc, po[:, CHUNK_OFF[c] + D: CHUNK_OFF[c] + D + 1]
                        )
                        nc.vector.tensor_scalar_mul(
                            st["out"][:, qi * D:(qi + 1) * D],
                            po[:, CHUNK_OFF[c]:CHUNK_OFF[c] + D],
                            rc,
                        )
                        nc.sync.dma_start(
                            out=out[h][:, qi * D:(qi + 1) * D],
                            in_=st["out"][:, qi * D:(qi + 1) * D],
                        )
                    return
                else:
                    # Drain po with one copy (frees both banks), then
                    # normalize on the otherwise-idle Pool engine so DVE
                    # stays free for the tri-masks on the critical path.
                    un = unpool.tile([128, 1024], F32, tag="un",
                                     name=f"un{h}_{qb}")
                    nc.vector.tensor_copy(un[:, 0:CHUNK_OFF[2] + VW],
                                          po[:, 0:CHUNK_OFF[2] + VW])
                    nc.vector.tensor_copy(un[:, 512:512 + VW],
                                          po[:, 512:512 + VW])
                    for c in range(4):
                        qi = qb * 4 + c
                        nc.gpsimd.normalize_recip(
                            st["out"][:, qi * D:(qi + 1) * D],
                            un[:, CHUNK_OFF[c]:CHUNK_OFF[c] + D],
                            un[:, CHUNK_OFF[c] + D: CHUNK_OFF[c] + D + 1],
                        )
                nc.sync.dma_start(
                    out=out[h][:, qb * 4 * D:(qb + 1) * 4 * D],
                    in_=st["out"][:, qb * 4 * D:(qb + 1) * 4 * D],
                )

            # ---- walk the global wave stream ----
            pi = 0
            for wi in range(len(cuts) - 1):
                lo, hi = cuts[wi], cuts[wi + 1]
                wsel = []
                while pi < len(gpieces):
                    h, j, qb, c0, nch, pos, sq = gpieces[pi]
                    g = h * head_cols + pos
                    if g >= hi:
                        break
                    wsel.append((pi, gpieces[pi]))
                    pi += 1
                # per-head exp segments [lo, hi) in head-local cols
                segs = []
                for _, (h, j, qb, c0, nch, pos, sq) in wsel:
                    g0, g1 = h * head_cols + pos, h * head_cols + pos + nch * 128
                    if segs and segs[-1][0] == h:
                        segs[-1][2] = pos + nch * 128
                    else:
                        segs.append([h, pos, pos + nch * 128])
                wv = {
                    "wi": wi, "lo": lo, "hi": hi,
                    "pieces": [p for _, p in wsel],
                    "pidx": [i for i, _ in wsel],
                    "segs": segs,
                }
                for _, (h, *_rest) in wsel:
                    if h not in head_st:
                        emit_head_dma(h)
                    # prefetch the next head a full head period ahead
                    if h + 1 < hpc and h + 1 not in head_st:
                        emit_head_dma(h + 1)
                emit_scores(wv)
                pending.append(wv)
                # shallower lag near the kernel tail so less PV work
                # remains after the final exp
                lag = 1 if wi >= len(cuts) - 3 else PV_LAG
                while len(pending) > lag:
                    emit_pv(pending.pop(0))
            for wv in pending:
                emit_pv(wv)
    nc.compile()
    return nc


def _pack_inputs(xq, xk, xv, s=S, b=B, h=H):
    """Full [B,S,H,D] fp32 inputs -> per-pair device layouts (bf16)."""
    bf16 = ml_dtypes.bfloat16
    nt = s // 128
    nh = b * h
    # [B,S,H,D] -> [B,H,S,D] -> [nh, S, D]
    q = np.transpose(np.asarray(xq), (0, 2, 1, 3)).reshape(nh, s, D)
    k = np.transpose(np.asarray(xk), (0, 2, 1, 3)).reshape(nh, s, D)
    v = np.transpose(np.asarray(xv), (0, 2, 1, 3)).reshape(nh, s, D)
    qT = np.ascontiguousarray(q.transpose(0, 2, 1)).astype(bf16)  # [nh, D, S]
    kT = np.ascontiguousarray(k.transpose(0, 2, 1)).astype(bf16)
    v4 = v.reshape(nh, nt, 128, D)
    ones = np.ones((nh, nt, 128, 1), np.float32)
    vA = np.concatenate([v4, ones], axis=3)          # [nh, nt, 128, VW]
    vA = np.ascontiguousarray(vA.transpose(0, 2, 1, 3)).reshape(nh, 128, nt * VW)
    vA = vA.astype(bf16)
    tri = np.triu(np.ones((128, 128), np.float32)).astype(bf16)
    return qT, kT, vA, tri


def _unpack_output(outs, s=S, b=B, h=H):
    """Per-core [hpc, 128, NT*D] bf16 -> [B, S, H*D] fp32."""
    nt = s // 128
    o = np.concatenate([np.asarray(x) for x in outs], axis=0)  # [nh, 128, nt*D]
    o = o.reshape(b * h, 128, nt, D).transpose(0, 2, 1, 3)     # [nh, nt, 128, D]
    o = o.reshape(b, h, s, D).transpose(0, 2, 1, 3)            # [B, S, H, D]
    return np.ascontiguousarray(o.reshape(b, s, h * D)).astype(np.float32)


_CACHE = {}


def _get_module():
    if "nc" not in _CACHE:
        _CACHE["nc"] = build_module()
    return _CACHE["nc"]


def make_in_maps(xq, xk, xv):
    qT, kT, vA, tri = _pack_inputs(xq, xk, xv)
    in_maps = []
    for core in range(N_CORES):
        sl = slice(core * HPC, (core + 1) * HPC)
        in_maps.append({
            "qT": np.ascontiguousarray(qT[sl]),
            "kT": np.ascontiguousarray(kT[sl]),
            "vA": np.ascontiguousarray(vA[sl]),
            "tri": tri,
        })
    return in_maps


def kernel(xq, xk, xv, cache_k, cache_v, mask, start_pos):
    assert int(start_pos) == 0, "kernel specialized for start_pos == 0"
    from concourse.bass_utils import run_bass_kernel_spmd

    nc = _get_module()
    in_maps = make_in_maps(xq, xk, xv)
    res = None
    for attempt in range(3):
        try:
            res = run_bass_kernel_spmd(nc, in_maps, core_ids=list(range(N_CORES)))
            break
        except Exception:
            if attempt == 2:
                raise
    outs = [res.results[i]["out"] for i in range(N_CORES)]
    return _unpack_output(outs)


# revision 25
# speedup vs baseline: 1.0142x; 1.0142x over previous
"""Causal multi-head attention (B=2, S=2048, H=32, D=128) on 8 TRN2 NeuronCores.

Strategy (tensor-parallel over (batch, head) pairs — 64 pairs, 8 per core):

Host side packs per-head inputs into device-friendly layouts:
  qT, kT : [hpc, D, S]  bf16 — Q^T / K^T per head (d on partitions)
  vA     : [hpc, 128, NT*129] bf16 — V tiled [kv-tile, 129] with a ones
           column appended (col 128) so the softmax denominator falls out of
           the PV matmul as an extra output column.
  tri    : [128, 128] bf16 — tri[p, f] = 1 iff p <= f (causal keep-mask for
           diagonal 128x128 blocks in S^T layout).

Device — one gapless bank-packed wave pipeline over the whole core:
  All causal S^T pieces (kv-tile x q-block, causally trimmed, split at
  512-col PSUM bank boundaries) of all 8 heads form a dense 272-bank
  stream, cut into uniform 3-bank (1536-col) waves; the only short waves
  (512 cols) are the very first and last, where the pipeline is filling or
  draining anyway.  Waves may straddle a head boundary (the exp is then
  split per head segment).  Per wave: QK matmuls into a double-buffered
  3-bank PSUM tile (PE, bf16), one exp per head-segment (ACT, scale folded
  in; no max subtraction — scores are O(5) so fp32 exp is safe) into a
  per-head pt tile in SBUF, diagonal squares fixed by a bf16 tri-mask
  multiply (DVE).  PV trails two waves behind with P^T chunks as the
  stationary operand, so output lands in [q, d] layout and the vA ones
  column accumulates row sums; per q-block the accumulator is drained
  (DVE) and normalized on the otherwise-idle Pool engine, then DMA'd out
  as bf16.

Uniform waves keep the PE->ACT 2-slot PSUM rotation cadence jitter-free:
ACT (the bottleneck at ~134us/core) sees near-identical 1536-col work
items back to back for the entire kernel.  Upper-triangle blocks are
skipped entirely: exp(-1e9) underflows to exactly 0.0 in fp32, so
dropping them is bit-equivalent to the reference softmax.
"""

import math

import numpy as np
import ml_dtypes

import concourse.bass as bass
import concourse.mybir as mybir
import concourse.tile as tile
from concourse import bacc
from concourse.tile_rust import add_dep_helper

B, S, H, D = 2, 2048, 32, 128
N_CORES = 8
HPC = (B * H) // N_CORES  # head-pairs per core
VW = D + 1                # V width including the ones column
SCALE = 1.0 / math.sqrt(D)
CHUNK_OFF = (0, 129, 258, 512)  # PV output chunk offsets (chunk 3 in bank 1)
BF16 = mybir.dt.bfloat16
F32 = mybir.dt.float32


def _head_layout(s=S):
    """Dense bank-packed causal S^T piece list for one head.

    Pieces are (j, qb, c0, nch, pos, square): kv-tile j of q-block qb,
    chunks c0..c0+nch-1 (128-col q-chunks within the q-block), packed at
    column pos.  `square` marks the piece whose first chunk is the diagonal
    square needing the tri mask.  Pieces never cross 512-col PSUM bank
    boundaries (split on the fly); widths/positions are all multiples of
    128 so chunks stay 128-aligned.
    """
    pieces = []
    pos = 0
    qnb = s // 512
    for qb in range(qnb):
        for j in range(4 * qb + 4):
            c0 = max(0, j - 4 * qb)
            nch = 4 - c0
            first = True
            while nch > 0:
                room = (512 - pos % 512) // 128
                n = min(nch, room)
                pieces.append((j, qb, c0, n, pos, first and c0 == j - 4 * qb))
                pos += n * 128
                c0 += n
                nch -= n
                first = False
    return pieces, pos


def build_module(hpc=HPC, s=S, wave_banks=3):
    nt = s // 128
    qnb = s // 512
    hpieces, head_cols = _head_layout(s)
    # Global piece stream: (h, j, qb, c0, nch, pos-within-head, square)
    gpieces = [(h, *p) for h in range(hpc) for p in hpieces]
    total_cols = hpc * head_cols
    # Wave cut points over global columns: short first wave (quick kernel
    # start), uniform 1536 interior waves, short remainder at the end.
    cuts = [0, 512]
    while cuts[-1] < total_cols:
        cuts.append(min(cuts[-1] + wave_banks * 512, total_cols))

    nc = bacc.Bacc(trn_type="TRN2")
    qT = nc.dram_tensor("qT", [hpc, D, s], BF16, kind="ExternalInput")
    kT = nc.dram_tensor("kT", [hpc, D, s], BF16, kind="ExternalInput")
    vA = nc.dram_tensor("vA", [hpc, 128, nt * VW], BF16, kind="ExternalInput")
    tri = nc.dram_tensor("tri", [128, 128], BF16, kind="ExternalInput")
    out = nc.dram_tensor("out", [hpc, 128, nt * D], BF16, kind="ExternalOutput")

    exp_fn = mybir.ActivationFunctionType.Exp

    last_piece = {}
    for idx, (h, j, qb, c0, nch, pos, sq) in enumerate(gpieces):
        last_piece[(h, qb)] = idx

    with tile.TileContext(nc) as tc:
        with (
            tc.tile_pool(name="const", bufs=1) as cpool,
            tc.tile_pool(name="io", bufs=2) as iopool,
            tc.tile_pool(name="pt", bufs=2) as ptpool,
            tc.tile_pool(name="ps", bufs=2, space="PSUM") as pspool,
            tc.tile_pool(name="po", bufs=1, space="PSUM") as popool,
            tc.tile_pool(name="nrm", bufs=4) as npool,
            tc.tile_pool(name="un", bufs=2) as unpool,
        ):
            tri_sb = cpool.tile([128, 128], BF16, tag="tri", name="tri_sb")
            head_st = {}
            qstate = {}
            pending = []   # wave dicts awaiting PV emission (lag queue)
            PV_LAG = 2     # PV trails scores by 2 waves: its exp/tri deps
                           # are complete by then (ps slot WAR), so PE never
                           # head-of-line blocks on ACT/DVE.

            def emit_head_dma(h):
                # Tiny first-wave slices first so head 0's first matmuls
                # start as soon as possible; 512-col slices keep later
                # waves' needs ahead of the bulk.
                kT_sb = iopool.tile([128, s], BF16, tag="kT", name=f"kT{h}")
                qT_sb = iopool.tile([128, s], BF16, tag="qT", name=f"qT{h}")
                vA_sb = iopool.tile([128, nt * VW], BF16, tag="vA",
                                    name=f"vA{h}")
                if h == 0:
                    # Parallel trigger engines: each dma_start costs ~600ns
                    # serial on its trigger engine's sequencer, so spreading
                    # the critical first slices across sync/gpsimd/vector
                    # gets wave 0/1 data in flight ~2us sooner.
                    nc.sync.dma_start(out=kT_sb[:, 0:512], in_=kT[h][:, 0:512])
                    nc.gpsimd.dma_start(out=qT_sb[:, 0:512],
                                        in_=qT[h][:, 0:512])
                    nc.gpsimd.dma_start(out=kT_sb[:, 512:1024],
                                        in_=kT[h][:, 512:1024])
                    nc.sync.dma_start(out=qT_sb[:, 512:1024],
                                      in_=qT[h][:, 512:1024])
                    nc.gpsimd.dma_start(out=vA_sb[:, 0:8 * VW],
                                        in_=vA[h][:, 0:8 * VW])
                    nc.scalar.dma_start(out=tri_sb, in_=tri[:, :])
                    nc.sync.dma_start(out=kT_sb[:, 1024:s],
                                      in_=kT[h][:, 1024:s])
                    nc.scalar.dma_start(out=qT_sb[:, 1024:s],
                                        in_=qT[h][:, 1024:s])
                    nc.gpsimd.dma_start(out=vA_sb[:, 8 * VW:],
                                        in_=vA[h][:, 8 * VW:])
                else:
                    nc.sync.dma_start(out=kT_sb, in_=kT[h])
                    nc.sync.dma_start(out=qT_sb, in_=qT[h])
                    nc.gpsimd.dma_start(out=vA_sb, in_=vA[h])
                out_sb = iopool.tile([128, nt * D], BF16, tag="osb",
                                     name=f"osb{h}")
                pt_sb = ptpool.tile([128, head_cols], BF16, tag="pt",
                                    name=f"pt{h}")
                head_st[h] = {"kT": kT_sb, "qT": qT_sb, "vA": vA_sb,
                              "out": out_sb, "pt": pt_sb}

            def emit_scores(wv):
                w0, w1 = wv["lo"], wv["hi"]
                ps = pspool.tile(
                    [128, wave_banks * 512], F32, tag="ps",
                    name=f"ps{wv['wi']}",
                )
                # High priority: the wave ACT consumes next must never
                # queue behind lagged-PV filler in the PE stream.
                with tc.high_priority(offset=150):
                    for (h, j, qb, c0, nch, pos, sq) in wv["pieces"]:
                        st = head_st[h]
                        w = nch * 128
                        q0 = qb * 512 + c0 * 128
                        g = h * head_cols + pos
                        nc.tensor.matmul(
                            ps[:, g - w0:g - w0 + w],
                            st["kT"][:, j * 128:(j + 1) * 128],
                            st["qT"][:, q0:q0 + w],
                            start=True, stop=True,
                        )
                # one exp per head segment of the wave
                for h, lo, hi in wv["segs"]:
                    nc.scalar.activation(
                        head_st[h]["pt"][:, lo:hi],
                        ps[:, h * head_cols + lo - w0:
                           h * head_cols + hi - w0],
                        exp_fn, scale=SCALE,
                    )
                for (h, j, qb, c0, nch, pos, sq) in wv["pieces"]:
                    if sq:  # diagonal square: causal mask
                        pt = head_st[h]["pt"]
                        nc.vector.tensor_mul(
                            pt[:, pos:pos + 128], pt[:, pos:pos + 128], tri_sb,
                        )

            def emit_pv(wv):
                for pi, (h, j, qb, c0, nch, pos, sq) in zip(
                        wv["pidx"], wv["pieces"]):
                    st = head_st[h]
                    key = (h, qb)
                    if key not in qstate:
                        # PV accumulator for this q-block.  start=True on
                        # the first write to each bank clears it bank-wide
                        # (has_written), so every other first write to the
                        # bank lands overwrite-where-clear and later ones
                        # accumulate.  Explicit deps pin the bank-0 clear
                        # before its sibling chunks' first writes.
                        qstate[key] = {
                            "po": popool.tile([128, 1024], F32, tag="po",
                                              name=f"po{h}_{qb}"),
                            "banks": set(), "clear": None, "written": set(),
                        }
                    q = qstate[key]
                    po = q["po"]
                    for c in range(c0, c0 + nch):
                        bank = 1 if c == 3 else 0
                        first_bank = bank not in q["banks"]
                        q["banks"].add(bank)
                        mm = nc.tensor.matmul(
                            po[:, CHUNK_OFF[c]:CHUNK_OFF[c] + VW],
                            st["pt"][:, pos + (c - c0) * 128:
                                     pos + (c - c0) * 128 + 128],
                            st["vA"][:, j * VW:(j + 1) * VW],
                            start=first_bank,
                            stop=False, skip_group_check=True,
                        )
                        if bank == 0:
                            if first_bank:
                                q["clear"] = mm.ins
                            elif c not in q["written"]:
                                add_dep_helper(
                                    mm.ins, q["clear"], sync=False,
                                    reason="bank0 clear before sibling writes")
                        q["written"].add(c)
                    if pi == last_piece[(h, qb)]:
                        emit_norm(h, qb, q["po"])

            def emit_norm(h, qb, po):
                st = head_st[h]
                if h == hpc - 1 and qb == qnb - 1:
                    # Kernel tail: normalize straight from PSUM on DVE
                    # (fast cadence, no drain — nothing reuses po after).
                    # Per-chunk out DMAs overlap each chunk's ~4us DMA
                    # trigger->transfer latency with the next normalize.
                    for c in range(4):
                        qi = qb * 4 + c
                        rc = npool.tile([128, 1], F32, tag="rc",
                                        name=f"rc{h}_{qi}")
                        nc.vector.reciprocal(
                            rc, po[:, CHUNK_OFF[c] + D: CHUNK_OFF[c] + D + 1]
                        )
                        nc.vector.tensor_scalar_mul(
                            st["out"][:, qi * D:(qi + 1) * D],
                            po[:, CHUNK_OFF[c]:CHUNK_OFF[c] + D],
                            rc,
                        )
                        if c == 2:
                            nc.sync.dma_start(
                                out=out[h][:, qb * 4 * D:qb * 4 * D + 3 * D],
                                in_=st["out"][:, qb * 4 * D:qb * 4 * D + 3 * D],
                            )
                    nc.sync.dma_start(
                        out=out[h][:, (qb * 4 + 3) * D:(qb + 1) * 4 * D],
                        in_=st["out"][:, (qb * 4 + 3) * D:(qb + 1) * 4 * D],
                    )
                    return
                else:
                    # Drain po with one copy (frees both banks), then
                    # normalize on the otherwise-idle Pool engine so DVE
                    # stays free for the tri-masks on the critical path.
                    un = unpool.tile([128, 1024], F32, tag="un",
                                     name=f"un{h}_{qb}")
                    nc.vector.tensor_copy(un[:, 0:CHUNK_OFF[2] + VW],
                                          po[:, 0:CHUNK_OFF[2] + VW])
                    nc.vector.tensor_copy(un[:, 512:512 + VW],
                                          po[:, 512:512 + VW])
                    for c in range(4):
                        qi = qb * 4 + c
                        nc.gpsimd.normalize_recip(
                            st["out"][:, qi * D:(qi + 1) * D],
                            un[:, CHUNK_OFF[c]:CHUNK_OFF[c] + D],
                            un[:, CHUNK_OFF[c] + D: CHUNK_OFF[c] + D + 1],
                        )
                nc.sync.dma_start(
                    out=out[h][:, qb * 4 * D:(qb + 1) * 4 * D],
                    in_=st["out"][:, qb * 4 * D:(qb + 1) * 4 * D],
                )

            # ---- walk the global wave stream ----
            pi = 0
            for wi in range(len(cuts) - 1):
                lo, hi = cuts[wi], cuts[wi + 1]
                wsel = []
                while pi < len(gpieces):
                    h, j, qb, c0, nch, pos, sq = gpieces[pi]
                    g = h * head_cols + pos
                    if g >= hi:
                        break
                    wsel.append((pi, gpieces[pi]))
                    pi += 1
                # per-head exp segments [lo, hi) in head-local cols
                segs = []
                for _, (h, j, qb, c0, nch, pos, sq) in wsel:
                    g0, g1 = h * head_cols + pos, h * head_cols + pos + nch * 128
                    if segs and segs[-1][0] == h:
                        segs[-1][2] = pos + nch * 128
                    else:
                        segs.append([h, pos, pos + nch * 128])
                wv = {
                    "wi": wi, "lo": lo, "hi": hi,
                    "pieces": [p for _, p in wsel],
                    "pidx": [i for i, _ in wsel],
                    "segs": segs,
                }
                for _, (h, *_rest) in wsel:
                    if h not in head_st:
                        emit_head_dma(h)
                    # prefetch the next head a full head period ahead
                    if h + 1 < hpc and h + 1 not in head_st:
                        emit_head_dma(h + 1)
                emit_scores(wv)
                pending.append(wv)
                # shallower lag near the kernel tail so less PV work
                # remains after the final exp
                lag = 1 if wi >= len(cuts) - 3 else PV_LAG
                while len(pending) > lag:
                    emit_pv(pending.pop(0))
            for wv in pending:
                emit_pv(wv)
    nc.compile()
    return nc


def _pack_inputs(xq, xk, xv, s=S, b=B, h=H):
    """Full [B,S,H,D] fp32 inputs -> per-pair device layouts (bf16)."""
    bf16 = ml_dtypes.bfloat16
    nt = s // 128
    nh = b * h
    # [B,S,H,D] -> [B,H,S,D] -> [nh, S, D]
    q = np.transpose(np.asarray(xq), (0, 2, 1, 3)).reshape(nh, s, D)
    k = np.transpose(np.asarray(xk), (0, 2, 1, 3)).reshape(nh, s, D)
    v = np.transpose(np.asarray(xv), (0, 2, 1, 3)).reshape(nh, s, D)
    qT = np.ascontiguousarray(q.transpose(0, 2, 1)).astype(bf16)  # [nh, D, S]
    kT = np.ascontiguousarray(k.transpose(0, 2, 1)).astype(bf16)
    v4 = v.reshape(nh, nt, 128, D)
    ones = np.ones((nh, nt, 128, 1), np.float32)
    vA = np.concatenate([v4, ones], axis=3)          # [nh, nt, 128, VW]
    vA = np.ascontiguousarray(vA.transpose(0, 2, 1, 3)).reshape(nh, 128, nt * VW)
    vA = vA.astype(bf16)
    tri = np.triu(np.ones((128, 128), np.float32)).astype(bf16)
    return qT, kT, vA, tri


def _unpack_output(outs, s=S, b=B, h=H):
    """Per-core [hpc, 128, NT*D] bf16 -> [B, S, H*D] fp32."""
    nt = s // 128
    o = np.concatenate([np.asarray(x) for x in outs], axis=0)  # [nh, 128, nt*D]
    o = o.reshape(b * h, 128, nt, D).transpose(0, 2, 1, 3)     # [nh, nt, 128, D]
    o = o.reshape(b, h, s, D).transpose(0, 2, 1, 3)            # [B, S, H, D]
    return np.ascontiguousarray(o.reshape(b, s, h * D)).astype(np.float32)


_CACHE = {}


def _get_module():
    if "nc" not in _CACHE:
        _CACHE["nc"] = build_module()
    return _CACHE["nc"]


def make_in_maps(xq, xk, xv):
    qT, kT, vA, tri = _pack_inputs(xq, xk, xv)
    in_maps = []
    for core in range(N_CORES):
        sl = slice(core * HPC, (core + 1) * HPC)
        in_maps.append({
            "qT": np.ascontiguousarray(qT[sl]),
            "kT": np.ascontiguousarray(kT[sl]),
            "vA": np.ascontiguousarray(vA[sl]),
            "tri": tri,
        })
    return in_maps


def kernel(xq, xk, xv, cache_k, cache_v, mask, start_pos):
    assert int(start_pos) == 0, "kernel specialized for start_pos == 0"
    from concourse.bass_utils import run_bass_kernel_spmd

    nc = _get_module()
    in_maps = make_in_maps(xq, xk, xv)
    res = None
    for attempt in range(3):
        try:
            res = run_bass_kernel_spmd(nc, in_maps, core_ids=list(range(N_CORES)))
            break
        except Exception:
            if attempt == 2:
                raise
    outs = [res.results[i]["out"] for i in range(N_CORES)]
    return _unpack_output(outs)
